# revision 57
# baseline (speedup 1.0000x reference)
"""CPC loss kernel for Trainium2 (8 NeuronCores, data-parallel over batch).

Contract: kernel(**inputs) takes the FULL unsharded inputs
(base_payload [128,512,128] f32, mapped_ctx_payload [128,512,128,4] f32,
seq_lens [128] i32, sample_ids [128,64] i32) and returns the scalar loss
as a 0-d float32 numpy array.

Design (v4 — length-specialized, slab DMAs, host lse assembly):
  - Rows are sorted by seq_len desc and dealt round-robin to the 8 cores;
    slot j on every core processes n = nch_sched[j] = max-over-cores
    ceil(len/128) chunks of 128 positions. The NEFF is compiled for the
    actual seq_lens (runtime specialization); positions beyond the last
    chunk contribute exactly ln(65) each, added on host.
  - The device does the only O(B*T*NNEG*E) work — the negative-logit
    matmuls — plus exp and the 64-way row sums:
      per slot, per (k,chunk): PE matmul ce_chunk.T @ negs -> psn[128,g,64]
      one ACT Exp(psn-40) -> bf16; Pool adds the two 32-halves; DVE
      reduces the remaining 32 -> rns[:, g].
    Inputs are two packed bf16 slabs (one ~large DMA per slot-pair):
      mce_slab [E, sum_j 4*n_j*128]  (ce, masked past len, (k,chunk)-major)
      neg_slab [E, 16*64]            (gathered negatives per row)
    Output: rns [128, G_total] f32 (exp'd-negative row sums; partition =
    position-within-chunk).
  - Host computes the positive logits (an einsum over data it already
    holds — 33M MACs, negligible vs the device's 2.1 GFLOP), assembles
    ln(exp(pos-40)+rns)+40-pos per position in f64, applies the
    1/(K*B*(T-i)) weights, and adds the skipped-position ln(65) constant.
"""

import os
import sys

import numpy as np

_TRN_REPO = "/opt/trn_rl_repo"
if _TRN_REPO not in sys.path:
    sys.path.insert(0, _TRN_REPO)

import ml_dtypes

BF16 = ml_dtypes.bfloat16
# fp8 for the mce/neg slabs halves the dominant DMA stream; the
# host-side lse assembly keeps pos exact, so only the negative logits
# see the extra quantization noise. e3m4: values are ~N(0,1) so 3
# exponent bits suffice, the extra mantissa bit halves the noise, and
# the PE runs float8e3 at 1 cycle/row (e4m3 takes 2).
FP8 = ml_dtypes.float8_e3m4
USE_FP8 = os.environ.get("KERNEL_FP8", "1") == "1"

B, T, E, K, NNEG = 128, 512, 128, 4, 64
NCORES = 8
BPC = B // NCORES  # batch rows (slots) per core
CHUNK = 128
SHIFT = 40.0  # logit shift before exp keeps Exp output in fp range

_compiled = None  # (key, nc) cache so repeated kernel() calls reuse the NEFF


def _schedule(seq_lens):
    """Sort rows by length desc, deal to (slot, core); per-slot chunk count
    is the max over the 8 cores so one NEFF serves all cores."""
    lens = np.asarray(seq_lens, dtype=np.int64)
    nch = -(-lens // CHUNK)  # ceil
    order = np.argsort(-lens, kind="stable")
    perm = order.reshape(BPC, NCORES)  # [slot, core] -> global row
    nch_sched = nch[perm].max(axis=1).astype(int)  # [BPC]
    return perm, nch_sched


def _layout(nch_sched):
    """Column offsets of each slot inside the packed mce slab and the
    group axis of the output."""
    mce_off, goff = [], []
    cm = cg = 0
    for n in nch_sched:
        mce_off.append(cm)
        goff.append(cg)
        cm += K * n * CHUNK
        cg += K * n
    return mce_off, cm, goff, cg


def _build_nc(nch_sched, iters=0, unroll=1):
    """iters=0: straight-line kernel. iters>0: body wrapped in a For_i loop
    (benchmarking only — amortizes host/RPC overhead across iterations)."""
    from contextlib import nullcontext

    from concourse import bacc, mybir, tile

    dt = mybir.dt
    f32 = dt.float32
    in_dt = dt.float8e3 if USE_FP8 else dt.bfloat16

    mce_off, C_mce, goff, GT = _layout(nch_sched)

    nc = bacc.Bacc(
        "TRN2", target_bir_lowering=False, debug=False, num_devices=NCORES
    )

    mce_d = nc.dram_tensor("mce", [E, C_mce], in_dt, kind="ExternalInput")
    neg_d = nc.dram_tensor("neg", [E, BPC * NNEG], in_dt, kind="ExternalInput")
    out_d = nc.dram_tensor("out", [E, GT], f32, kind="ExternalOutput")

    with tile.TileContext(nc) as tc:
        with (
            tc.tile_pool(name="const", bufs=1) as p_const,
            tc.tile_pool(name="expd", bufs=6) as p_expd,
            tc.tile_pool(name="exph", bufs=6) as p_exph,
            tc.tile_pool(name="neg", bufs=2) as p_neg,
            tc.tile_pool(name="ps", bufs=4, space="PSUM") as p_ps,
        ):
            shift_t = p_const.tile([E, 1], f32, tag="shift")
            nc.vector.memset(shift_t[:], -SHIFT)
            mce_t = p_const.tile([E, C_mce], in_dt, tag="mce")
            out_t = p_const.tile([E, GT], f32, tag="out")

            loop_cm = tc.For_i(0, iters, 1) if iters else nullcontext()
            with loop_cm:
                for _rep in range(unroll if iters else 1):
                    _emit_body(
                        nc, mybir, nch_sched, mce_off, C_mce, goff, GT,
                        p_expd, p_exph, p_ps,
                        mce_d, neg_d, out_d,
                        shift_t, mce_t, p_neg, out_t,
                    )

    nc.compile()
    return nc


def _emit_body(
    nc, mybir, nch_sched, mce_off, C_mce, goff, GT,
    p_expd, p_exph, p_ps,
    mce_d, neg_d, out_d,
    shift_t, mce_t, p_neg, out_t,
):
    AX = mybir.AxisListType
    ALU = mybir.AluOpType
    ACT = mybir.ActivationFunctionType
    bf16 = mybir.dt.bfloat16

    nslots = len(nch_sched)
    mce_end = mce_off[1:] + [C_mce]

    # negatives double-buffered across loop iterations: a single tile
    # is read by every matmul, so iteration i+1's neg DMA would WAR-wait
    # on iteration i's LAST matmul, serializing the whole pipeline
    neg_t = p_neg.tile([E, BPC * NNEG], mce_t.dtype, tag="neg")

    # ABLATE: stage-isolation for HW timing (dma < mm < exp < full).
    # Ablated variants produce garbage results; only timing is valid.
    ablate = os.environ.get("ABLATE", "")
    stage = {"dma": 0, "mm": 1, "exp": 2}.get(ablate, 3)
    if stage < 3:
        nc.vector.memset(out_t[:], 0.0)

    # input DMAs: half of slot 0 first so the first matmuls start ASAP,
    # then the negatives (every matmul needs them), then the rest of the
    # mce slab one slot-pair at a time so compute overlaps the stream.
    # (All on the sync queue: spreading issue across the scalar HWDGE
    # queue or going per-slot measured WORSE on HW — ACT-sequencer DMA
    # work delays the exp ladder, and 2x the issues raises HWDGE busy.)
    half0 = mce_off[0] + 2 * int(nch_sched[0]) * CHUNK
    nc.sync.dma_start(out=mce_t[:, :half0], in_=mce_d[:, :half0])
    nc.sync.dma_start(out=neg_t[:], in_=neg_d[:])
    nc.sync.dma_start(
        out=mce_t[:, half0 : mce_end[0]], in_=mce_d[:, half0 : mce_end[0]]
    )
    nc.sync.dma_start(
        out=mce_t[:, mce_end[0] : mce_end[1]],
        in_=mce_d[:, mce_end[0] : mce_end[1]],
    )
    step = 4 if os.environ.get("KERNEL_DMAQUAD", "0") == "1" else 2
    for j0 in range(2, nslots, step):
        m_lo = mce_off[j0]
        m_hi = mce_end[min(j0 + step, nslots) - 1]
        nc.sync.dma_start(out=mce_t[:, m_lo:m_hi], in_=mce_d[:, m_lo:m_hi])

    # Row-sum strategy: KERNEL_SUM=2stage (default) does a DVE bf16 add
    # of the 32-halves in 2x mode followed by a 32-wide 1x reduce — 25%
    # less DVE busy than KERNEL_SUM=direct's single 64-wide reduce.
    # (A GpSimd offload of the add was measured ~3 us WORSE on HW: Q7
    # software-op cost far exceeds its cost-model estimate.)
    two_stage = os.environ.get("KERNEL_SUM", "2stage") == "2stage"
    split = [two_stage] * nslots

    for j in range(nslots):
        n = int(nch_sched[j])
        G = K * n
        W = n * CHUNK
        moff = mce_off[j]
        g0 = goff[j]

        if stage < 1:
            continue
        psn = p_ps.tile([E, 16, NNEG], mybir.dt.float32, tag="psn")
        expn = p_expd.tile([E, 16, NNEG], bf16, tag="expn")
        # (Splitting slot 0's exp into two halves to start the ACT ladder
        # ~2 us earlier measured neutral-to-worse on HW — the extra op +
        # semaphore hops offset the earlier start. Keep one exp per slot.)
        kcuts = (K,)
        k_lo = 0
        for k_hi in kcuts:
            for k in range(k_lo, k_hi):
                for c in range(n):
                    g = k * n + c
                    col = k * W + c * CHUNK
                    nc.tensor.matmul(
                        psn[:, g, :],
                        lhsT=mce_t[:, moff + col : moff + col + CHUNK],
                        rhs=neg_t[:, j * NNEG : (j + 1) * NNEG],
                        start=True,
                        stop=True,
                    )
            if stage >= 2:
                nc.scalar.activation(
                    expn[:, k_lo * n : k_hi * n, :],
                    psn[:, k_lo * n : k_hi * n, :],
                    ACT.Exp,
                    bias=shift_t[:],
                )
            k_lo = k_hi

        if stage < 2:
            continue
        if stage < 3:
            continue
        if split[j]:
            # two-stage on DVE: the bf16 add of the 32-halves runs in 2x
            # mode, the remaining 32-wide reduce at 1x — 25% less DVE
            # busy than a direct 64-wide reduce (which has no 2x mode).
            exph = p_exph.tile([E, 16, NNEG // 2], bf16, tag="exph")
            nc.vector.tensor_add(
                exph[:, :G, :],
                expn[:, :G, 0 : NNEG // 2],
                expn[:, :G, NNEG // 2 : NNEG],
            )
            nc.vector.tensor_reduce(
                out_t[:, g0 : g0 + G], exph[:, :G, :], axis=AX.X,
                op=ALU.add,
            )
        else:
            nc.vector.tensor_reduce(
                out_t[:, g0 : g0 + G], expn[:, :G, :], axis=AX.X,
                op=ALU.add,
            )
        if j == nslots - 6:
            # early flush: ship the finished front of out_t while the
            # short tail slots are still reducing
            gh = goff[j + 1]
            nc.sync.dma_start(out=out_d[:, :gh], in_=out_t[:, :gh])

    gh = goff[nslots - 5]
    nc.sync.dma_start(out=out_d[:, gh:], in_=out_t[:, gh:])


def _mask_mce(mapped_ctx_payload, seq_lens):
    mce = np.asarray(mapped_ctx_payload, dtype=np.float32)
    lens = np.asarray(seq_lens, dtype=np.int64)
    mask = (np.arange(T)[None, :] < lens[:, None]).astype(np.float32)
    return mce * mask[:, :, None, None]


def _prep_inputs(base_payload, mce_masked, sample_ids, perm, nch_sched):
    base = np.asarray(base_payload, dtype=np.float32)
    sids = np.asarray(sample_ids, dtype=np.int64)

    mce_off, C_mce, goff, GT = _layout(nch_sched)

    # negatives gathered from the flattened pool: [B, 64, E] -> [B, E, 64]
    negs = base.reshape(B * T, E)[sids].transpose(0, 2, 1)

    in_dt = FP8 if USE_FP8 else BF16
    in_maps = []
    for core in range(NCORES):
        mce_slab = np.zeros((E, C_mce), dtype=in_dt)
        neg_slab = np.zeros((E, BPC * NNEG), dtype=in_dt)
        for j in range(BPC):
            r = int(perm[j, core])
            n = int(nch_sched[j])
            W = n * CHUNK
            # ce [E, K, W], masked past len (mask already applied), padded
            # with zeros past T when W > T
            ceT = mce_masked[r].transpose(1, 2, 0)  # [E, K, T]
            ce = np.zeros((E, K, W), dtype=np.float32)
            w_real = min(W, T)
            ce[:, :, :w_real] = ceT[:, :, :w_real]
            mce_slab[:, mce_off[j] : mce_off[j] + K * W] = (
                ce.reshape(E, K * W).astype(in_dt)
            )
            neg_slab[:, j * NNEG : (j + 1) * NNEG] = negs[r].astype(in_dt)
        in_maps.append({"mce": mce_slab, "neg": neg_slab})
    return in_maps


def _pos_host(base_payload, mce_masked):
    """Positive logits pos[b, k, s] = sum_e ce_k[b,s,e] * base[b,s+k+1,e],
    zero past seq_len (ce is masked). Positions s >= T-(k+1) are unused."""
    base = np.asarray(base_payload, dtype=np.float32)
    pos = np.zeros((B, K, T), dtype=np.float64)
    for k in range(K):
        i = k + 1
        pos[:, k, : T - i] = np.einsum(
            "bse,bse->bs", mce_masked[:, : T - i, :, k], base[:, i:, :]
        )
    return pos


def _combine(results, pos_all, seq_lens, perm, nch_sched):
    mce_off, C_mce, goff, GT = _layout(nch_sched)
    ln65 = float(np.log(65.0))
    total = 0.0
    for core in range(NCORES):
        rns = np.asarray(results[core]["out"], dtype=np.float64)
        for j in range(BPC):
            r = int(perm[j, core])
            n = int(nch_sched[j])
            W = n * CHUNK
            for k in range(K):
                i = k + 1
                w = 1.0 / (K * B * (T - i))
                g0 = goff[j] + k * n
                # element (p, c) -> position s = c*128 + p
                rn = rns[:, g0 : g0 + n].T.reshape(-1)  # [W]
                nv = min(W, T - i)  # computed positions in the loss
                pos = pos_all[r, k, :nv]
                term = (
                    np.log(np.exp(pos - SHIFT) + rn[:nv]) + SHIFT - pos
                )
                total += w * (term.sum() + ln65 * max(0, (T - i) - W))
    return np.float32(total)


_last_results = None
_last_exec_time_ns = None


def kernel(base_payload, mapped_ctx_payload, seq_lens, sample_ids):
    global _compiled, _last_results, _last_exec_time_ns
    from concourse.bass_utils import run_bass_kernel_spmd

    perm, nch_sched = _schedule(seq_lens)
    key = (USE_FP8,) + tuple(int(x) for x in nch_sched)
    if _compiled is None or _compiled[0] != key:
        _compiled = (key, _build_nc(nch_sched))
    nc = _compiled[1]

    mce_masked = _mask_mce(mapped_ctx_payload, seq_lens)
    in_maps = _prep_inputs(
        base_payload, mce_masked, sample_ids, perm, nch_sched
    )
    pos_all = _pos_host(base_payload, mce_masked)
    trace = bool(int(os.environ.get("KERNEL_TRACE", "0")))
    res = run_bass_kernel_spmd(nc, in_maps, list(range(NCORES)), trace=trace)
    _last_results = res
    _last_exec_time_ns = res.exec_time_ns
    return _combine(res.results, pos_all, np.asarray(seq_lens), perm, nch_sched)
